# revision 26
# baseline (speedup 1.0000x reference)
"""BandSplit Trainium2 kernel: 8-core data-parallel over batch.

out[b,t,n,d] = rsqrt(ms + eps) * (x_band @ (norm_w * W)) + bias
with ms = sum(x_band^2)/(4*fn),  x_band = contiguous freq slices of X.

Math restructure (exact):
  rsqrt(ms + eps) = sqrt(4fn) / sqrt(ssum),  ssum = sum(x^2) + 4*fn*eps
  out = (1/sqrt(ssum)) * (x @ W2 + sqrt(ssum) * b)     [bias-row trick]
  where W2[n,p,:] = sqrt(4fn) * norm_w[n,p] * W[n,p,:]

Per core (one batch element, T=512 tokens, 4 passes of 128):
  1. DMA X planes into natural layout (t part, f free).
  2. ssum per band via one fused multiply-reduce per band (eps as initial).
  3. Free-axis gather (on GpSimd) rearranges columns into the packed
     band-major row order; bands grouped by equal width give affine 3D
     APs, one copy per (plane, width-group). Bias slots get sqrt(ssum).
  4. PE-transpose each 128-column block -> packed row chunks (XtB, f32r).
  5. Per band: 1-2 float32r matmuls (K=4fn+1, M=128 tokens, N=384).
  6. Evict PSUM->SBUF scaled by 1/sqrt(ssum) per token; DMA out.
"""

import math
import numpy as np

# ---------------- problem geometry (hardcoded, matches reference) ----------
SR, N_FFT, D = 44100, 2048, 384
RANGES = [(1000, 2), (2000, 4), (4000, 12), (8000, 24), (16000, 48)]


def _compute_bands(sr=SR, n_fft=N_FFT):
    hz_per_bin = sr / n_fft
    max_bin = n_fft // 2 + 1
    boundaries = [0]
    for hi_hz, bins in RANGES:
        hi_bin = math.floor(hi_hz / hz_per_bin)
        while boundaries[-1] + bins <= hi_bin and boundaries[-1] + bins <= max_bin:
            boundaries.append(boundaries[-1] + bins)
    if boundaries[-1] < max_bin:
        remaining = max_bin - boundaries[-1]
        step = math.ceil(remaining / 6)
        b = boundaries[-1]
        while b + step < max_bin:
            b += step
            boundaries.append(b)
        boundaries.append(max_bin)
    return [(boundaries[i], boundaries[i + 1]) for i in range(len(boundaries) - 1)]


BANDS = _compute_bands()
N_BANDS = len(BANDS)
assert N_BANDS == 62
FN = [r - l for l, r in BANDS]
MAXF = max(FN)
F_BINS = N_FFT // 2 + 1  # 1025
EPS = 1e-8
B, C, T = 8, 2, 512
TT = 128  # tokens per pass (matmul M)
NP = T // TT  # 4 passes
NPLANE = 4  # (c, ri) combinations, g = 2*c + ri

# ---------------- regular row layout by equal-width band groups ------------
# logical row j of band n: j=0 -> bias; j=1+g*fn+k -> plane g, freq l+k.
# Bands with equal fn are consecutive; within a group each band's rows
# start at G_base + i*pad, giving affine gather patterns.


def _pad_for(rows):
    for p in (32, 64, 128, 256):
        if rows <= p:
            return p
    raise AssertionError


def _plan():
    groups = []  # (n0, k, fn, l0, pad, gbase)
    rowbase = 0
    n = 0
    while n < N_BANDS:
        fn = FN[n]
        k = 1
        while n + k < N_BANDS and FN[n + k] == fn:
            k += 1
        rows = 1 + 4 * fn
        pad = _pad_for(rows)
        gbase = rowbase
        rowbase += ((k * pad + 127) // 128) * 128
        groups.append((n, k, fn, BANDS[n][0], pad, gbase))
        n += k
    nrows = rowbase  # multiple of 128
    nchunk = nrows // 128

    band_base = {}
    for (n0, k, fn, l0, pad, gbase) in groups:
        for i in range(k):
            band_base[n0 + i] = gbase + i * pad

    # matmul segments per band: (chunk, row0, klen) covering 1+4fn rows
    segs0 = []
    for n in range(N_BANDS):
        rows = 1 + 4 * FN[n]
        bb = band_base[n]
        out = []
        while rows > 0:
            ch, r0 = bb // 128, bb % 128
            kl = min(rows, 128 - r0)
            out.append((ch, r0, kl))
            bb += kl
            rows -= kl
        segs0.append(out)

    # W2 column blocks: greedy interval packing of (row0, row0+klen),
    # largest-first so full-height segments claim blocks before slivers.
    allsegs = []
    for n in range(N_BANDS):
        for si, (ch, r0, kl) in enumerate(segs0[n]):
            allsegs.append((kl, n, si, ch, r0))
    allsegs.sort(key=lambda x: -x[0])
    colblocks = []
    cb_of = {}
    for (kl, n, si, ch, r0) in allsegs:
        for cbi in range(len(colblocks) + 1):
            if cbi == len(colblocks):
                colblocks.append([])
            ivs = colblocks[cbi]
            if all(e <= r0 or s >= r0 + kl for (s, e) in ivs):
                ivs.append((r0, r0 + kl))
                cb_of[(n, si)] = cbi
                break
    ncolb = len(colblocks)
    segs = []
    for n in range(N_BANDS):
        segs.append(
            [
                (ch, r0, kl, cb_of[(n, si)])
                for si, (ch, r0, kl) in enumerate(segs0[n])
            ]
        )

    # host W2 packing map: logical row j of band n -> (w2row, colblock)
    w2map = []
    for n in range(N_BANDS):
        rows = 1 + 4 * FN[n]
        m = []
        j = 0
        for (ch, r0, kl, cb) in segs[n]:
            for q in range(kl):
                m.append((r0 + q, cb))
            j += kl
        assert j == rows
        w2map.append(m)
    return groups, nchunk, ncolb, segs, band_base, w2map


GROUPS, NCHUNK, NCOLB, SEGS, BAND_BASE, W2MAP = _plan()
NROWS = NCHUNK * 128
CHUNK_GROUP = []
for ch in range(NCHUNK):
    gi = max(i for i, g in enumerate(GROUPS) if g[5] <= ch * 128)
    CHUNK_GROUP.append(gi)

# ---------------- host-side constant prep ---------------------------------


def _prep_consts(norm_w, W, b):
    w2sb = np.zeros((128, NCOLB * D), np.float32)
    for n in range(N_BANDS):
        fn = FN[n]
        s = math.sqrt(4.0 * fn)
        row, cb = W2MAP[n][0]
        w2sb[row, cb * D:(cb + 1) * D] = b[n]
        w2rows = (s * norm_w[n][:, None] * W[n]).astype(np.float32)  # (216, 384)
        for g in range(NPLANE):
            for k in range(fn):
                row, cb = W2MAP[n][1 + g * fn + k]
                w2sb[row, cb * D:(cb + 1) * D] = w2rows[g * MAXF + k]
    idt = np.eye(128, dtype=np.float32)
    epsc = np.broadcast_to(
        (4.0 * np.asarray(FN, np.float64) * EPS).astype(np.float32)[None, :],
        (128, N_BANDS),
    ).copy()
    return w2sb, idt, epsc


# ---------------- bass kernel builder -------------------------------------

_BUILT = {}


def _build():
    if "nc" in _BUILT:
        return _BUILT["nc"]
    from contextlib import ExitStack
    import concourse.bacc as bacc
    import concourse.mybir as mybir
    from concourse import tile

    f32 = mybir.dt.float32
    f32r = mybir.dt.float32r

    nc = bacc.Bacc(None, target_bir_lowering=False)
    x_re = nc.declare_dram_parameter("X_real", [C, T, F_BINS], f32, isOutput=False)
    x_im = nc.declare_dram_parameter("X_imag", [C, T, F_BINS], f32, isOutput=False)
    w2_e = nc.declare_dram_parameter("W2", [128, NCOLB * D], f32r, isOutput=False)
    id_e = nc.declare_dram_parameter("IDT", [128, 128], f32, isOutput=False)
    eps_e = nc.declare_dram_parameter("EPSC", [128, N_BANDS], f32, isOutput=False)
    out_e = nc.declare_dram_parameter("out", [T, N_BANDS, D], f32, isOutput=True)

    GSZ = 8   # bands per output staging group
    WLD = 2   # W2 colblocks per staged load chunk

    with tile.TileContext(nc) as tc, ExitStack() as ctx:
        const = ctx.enter_context(tc.tile_pool(name="const", bufs=1))
        xtbp = ctx.enter_context(tc.tile_pool(name="xtbp", bufs=1))
        x4p = ctx.enter_context(tc.tile_pool(name="x4p", bufs=2))
        msp = ctx.enter_context(tc.tile_pool(name="msv", bufs=1))
        scr = ctx.enter_context(tc.tile_pool(name="scr", bufs=1))
        spool = ctx.enter_context(tc.tile_pool(name="stagep", bufs=2))
        trps = ctx.enter_context(tc.tile_pool(name="trp", bufs=2, space="PSUM"))
        mmps = ctx.enter_context(tc.tile_pool(name="mmp", bufs=6, space="PSUM"))

        idsb = const.tile([128, 128], f32)
        nc.sync.dma_start(out=idsb[:], in_=id_e[:])
        epsc = const.tile([128, N_BANDS], f32)
        nc.sync.dma_start(out=epsc[:], in_=eps_e[:])
        w2sb = const.tile([128, NCOLB * D], f32r)

        # double-buffered per-group gather tiles
        xcat = [[], []]
        for bi in range(2):
            for gi, (n0, k, fn, l0, pad, gbase) in enumerate(GROUPS):
                gw = (GROUPS[gi + 1][5] - gbase) if gi + 1 < len(GROUPS) else (
                    NROWS - gbase
                )
                xg = const.tile(
                    [128, gw], f32, name=f"xcat{bi}_{gi}", tag=f"xcat{bi}_{gi}"
                )
                xcat[bi].append(xg)
                nc.gpsimd.memset(xg[:], 0.0)
        xtb = [
            xtbp.tile([128, TT], f32r, name=f"xtb{m}", tag=f"xtb{m}")
            for m in range(NCHUNK)
        ]

        def phase_a(ps, gather_eng=None):
            """input DMA, band sums, sqrt, gathers + bias into xcat[ps%2]."""
            t0 = ps * TT
            xb = xcat[ps % 2]
            x4 = [
                x4p.tile([128, F_BINS], f32, tag=f"x4_{g}", name=f"x4_{g}")
                for g in range(NPLANE)
            ]
            qsum = scr.tile([128, F_BINS], f32, tag="qsum")
            sq_b = scr.tile([128, F_BINS], f32, tag="sqb")
            for g in range(NPLANE):
                xsrc = x_re if g % 2 == 0 else x_im
                nc.sync.dma_start(out=x4[g][:], in_=xsrc[g // 2, t0:t0 + TT, :])
                dst = qsum if g == 0 else sq_b
                nc.vector.tensor_tensor(
                    out=dst[:], in0=x4[g][:], in1=x4[g][:],
                    op=mybir.AluOpType.mult,
                )
                if g > 0:
                    nc.vector.tensor_tensor(
                        out=qsum[:], in0=qsum[:], in1=sq_b[:],
                        op=mybir.AluOpType.add,
                    )
            ssum_raw = msp.tile([128, N_BANDS], f32, tag="ssumr")
            for (n0, k, fn, l0, pad, gbase) in GROUPS:
                nc.vector.tensor_reduce(
                    out=ssum_raw[:, n0:n0 + k].rearrange("p (k o) -> p k o", o=1),
                    in_=qsum[:, l0:l0 + k * fn].rearrange("p (k f) -> p k f", k=k),
                    op=mybir.AluOpType.add,
                    axis=mybir.AxisListType.X,
                )
            ssum = msp.tile([128, N_BANDS], f32, tag="ssum")
            nc.vector.tensor_tensor(
                out=ssum[:], in0=ssum_raw[:], in1=epsc[:],
                op=mybir.AluOpType.add,
            )
            sqrt_n = msp.tile([128, N_BANDS], f32, tag="sqrtn")
            nc.scalar.activation(
                out=sqrt_n[:], in_=ssum[:], func=mybir.ActivationFunctionType.Sqrt
            )
            rs = msp.tile([128, N_BANDS], f32, tag=f"rs{ps}")
            nc.vector.reciprocal(rs[:], sqrt_n[:])
            geng = gather_eng or nc.gpsimd
            for gi, (n0, k, fn, l0, pad, gbase) in enumerate(GROUPS):
                xg = xb[gi]
                for g in range(NPLANE):
                    src = x4[g][:, l0:l0 + k * fn].rearrange(
                        "p (k f) -> p k f", k=k
                    )
                    dst = xg[:, 0:k * pad].rearrange(
                        "p (k q) -> p k q", k=k
                    )[:, :, 1 + g * fn:1 + (g + 1) * fn]
                    geng.tensor_copy(dst, src)
                dstb = xg[:, 0:k * pad].rearrange(
                    "p (k q) -> p k q", k=k
                )[:, :, 0:1]
                geng.tensor_copy(
                    dstb, sqrt_n[:, n0:n0 + k].rearrange("p (k o) -> p k o", o=1)
                )
            return rs

        def phase_b(ps, rs):
            """transposes, per-band matmuls, scaled eviction, output DMA."""
            t0 = ps * TT
            xb = xcat[ps % 2]
            for ch in range(NCHUNK):
                gi = CHUNK_GROUP[ch]
                off = ch * 128 - GROUPS[gi][5]
                ptr = trps.tile([128, 128], f32, tag="trp")
                nc.tensor.transpose(ptr[:], xb[gi][:, off:off + 128], idsb[:])
                if ch % 2 == 0:
                    nc.vector.tensor_copy(xtb[ch][:], ptr[:])
                else:
                    nc.scalar.copy(xtb[ch][:], ptr[:])
            for n0 in range(0, N_BANDS, GSZ):
                gn = min(GSZ, N_BANDS - n0)
                stage = spool.tile([128, GSZ * D], f32, tag="stage")
                for n in range(n0, n0 + gn):
                    pmm = mmps.tile([128, D], f32, tag="mmp")
                    nseg = len(SEGS[n])
                    for si, (ch, row0, klen, cb) in enumerate(SEGS[n]):
                        nc.tensor.matmul(
                            pmm[:],
                            lhsT=xtb[ch][row0:row0 + klen, :],
                            rhs=w2sb[row0:row0 + klen, cb * D:(cb + 1) * D],
                            start=(si == 0),
                            stop=(si == nseg - 1),
                            tile_position=(row0, 0),
                        )
                    slot = stage[:, (n - n0) * D:(n - n0 + 1) * D]
                    if n % 2 == 0:
                        nc.vector.tensor_scalar_mul(slot, pmm[:], rs[:, n:n + 1])
                    else:
                        nc.scalar.mul(slot, pmm[:], rs[:, n:n + 1])
                nc.sync.dma_start(
                    out=out_e[t0:t0 + TT, n0:n0 + gn, :],
                    in_=stage[:, 0:gn * D].rearrange("p (n d) -> p n d", n=gn),
                )

        def load_w2():
            nc.sync.dma_start(out=w2sb[:], in_=w2_e[:])

        # software pipeline: A(0) w2 A(1) B(0) A(2) B(1) A(3) B(2) B(3)
        rs_of = {}
        rs_of[0] = phase_a(0, gather_eng=nc.vector)
        load_w2()
        rs_of[1] = phase_a(1)
        phase_b(0, rs_of[0])
        rs_of[2] = phase_a(2)
        phase_b(1, rs_of[1])
        rs_of[3] = phase_a(3)
        phase_b(2, rs_of[2])
        phase_b(3, rs_of[3])

    nc.finalize()
    _BUILT["nc"] = nc
    return nc


# ---------------- entry points --------------------------------------------


def _run(in_maps, trace=False):
    from concourse.bass_utils import run_bass_kernel_spmd

    nc = _build()
    return run_bass_kernel_spmd(nc, in_maps, core_ids=list(range(8)), trace=trace)


def _run_traced(in_maps, tmpdir=None):
    from concourse.bass_utils import run_bass_kernel_spmd

    nc = _build()
    return run_bass_kernel_spmd(
        nc, in_maps, core_ids=list(range(8)), trace=True, tmpdir=tmpdir
    )


def _make_in_maps(X_real, X_imag, norm_w, W, b):
    X_real = np.ascontiguousarray(np.asarray(X_real, np.float32))
    X_imag = np.ascontiguousarray(np.asarray(X_imag, np.float32))
    w2sb, idt, epsc = _prep_consts(
        np.asarray(norm_w, np.float32), np.asarray(W, np.float32),
        np.asarray(b, np.float32),
    )
    return [
        {
            "X_real": X_real[i],
            "X_imag": X_imag[i],
            "W2": w2sb,
            "IDT": idt,
            "EPSC": epsc,
        }
        for i in range(B)
    ]


def kernel(X_real, X_imag, norm_w, W, b):
    res = _run(_make_in_maps(X_real, X_imag, norm_w, W, b), trace=False)
    return np.stack([res.results[i]["out"] for i in range(B)]).astype(np.float32)


def kernel_profiled(X_real, X_imag, norm_w, W, b):
    res = _run(_make_in_maps(X_real, X_imag, norm_w, W, b), trace=True)
    out = np.stack([res.results[i]["out"] for i in range(B)]).astype(np.float32)
    return out, res


if __name__ == "__main__":
    print(f"NCHUNK={NCHUNK} NCOLB={NCOLB} NROWS={NROWS}")
    print(f"groups: {[(g[0], g[1], g[2], g[4]) for g in GROUPS]}")
    print(f"matmul segs per pass: {sum(len(s) for s in SEGS)}")
    per_part = (2 * NROWS * 4 + NCHUNK * TT * 4 + NCOLB * D * 4
                + 2 * NPLANE * F_BINS * 4 + 2 * 4 * D * 4 + 3 * 4 * D * 4) / 1024
    print(f"approx SBUF per partition: {per_part:.0f} KB")


# revision 27
# speedup vs baseline: 1.0595x; 1.0595x over previous
"""BandSplit Trainium2 kernel: 8-core data-parallel over batch.

out[b,t,n,d] = rsqrt(ms + eps) * (x_band @ (norm_w * W)) + bias
with ms = sum(x_band^2)/(4*fn),  x_band = contiguous freq slices of X.

Math restructure (exact):
  rsqrt(ms + eps) = sqrt(4fn) / sqrt(ssum),  ssum = sum(x^2) + 4*fn*eps
  out = (1/sqrt(ssum)) * (x @ W2 + sqrt(ssum) * b)     [bias-row trick]
  where W2[n,p,:] = sqrt(4fn) * norm_w[n,p] * W[n,p,:]

Per core (one batch element, T=512 tokens, 4 passes of 128):
  1. DMA X planes into natural layout (t part, f free).
  2. ssum per band via one fused multiply-reduce per band (eps as initial).
  3. Free-axis gather (on GpSimd) rearranges columns into the packed
     band-major row order; bands grouped by equal width give affine 3D
     APs, one copy per (plane, width-group). Bias slots get sqrt(ssum).
  4. PE-transpose each 128-column block -> packed row chunks (XtB, f32r).
  5. Per band: 1-2 float32r matmuls (K=4fn+1, M=128 tokens, N=384).
  6. Evict PSUM->SBUF scaled by 1/sqrt(ssum) per token; DMA out.
"""

import math
import numpy as np

# ---------------- problem geometry (hardcoded, matches reference) ----------
SR, N_FFT, D = 44100, 2048, 384
RANGES = [(1000, 2), (2000, 4), (4000, 12), (8000, 24), (16000, 48)]


def _compute_bands(sr=SR, n_fft=N_FFT):
    hz_per_bin = sr / n_fft
    max_bin = n_fft // 2 + 1
    boundaries = [0]
    for hi_hz, bins in RANGES:
        hi_bin = math.floor(hi_hz / hz_per_bin)
        while boundaries[-1] + bins <= hi_bin and boundaries[-1] + bins <= max_bin:
            boundaries.append(boundaries[-1] + bins)
    if boundaries[-1] < max_bin:
        remaining = max_bin - boundaries[-1]
        step = math.ceil(remaining / 6)
        b = boundaries[-1]
        while b + step < max_bin:
            b += step
            boundaries.append(b)
        boundaries.append(max_bin)
    return [(boundaries[i], boundaries[i + 1]) for i in range(len(boundaries) - 1)]


BANDS = _compute_bands()
N_BANDS = len(BANDS)
assert N_BANDS == 62
FN = [r - l for l, r in BANDS]
MAXF = max(FN)
F_BINS = N_FFT // 2 + 1  # 1025
EPS = 1e-8
B, C, T = 8, 2, 512
TT = 128  # tokens per pass (matmul M)
NP = T // TT  # 4 passes
NPLANE = 4  # (c, ri) combinations, g = 2*c + ri

# ---------------- regular row layout by equal-width band groups ------------
# logical row j of band n: j=0 -> bias; j=1+g*fn+k -> plane g, freq l+k.
# Bands with equal fn are consecutive; within a group each band's rows
# start at G_base + i*pad, giving affine gather patterns.


def _pad_for(rows):
    for p in (32, 64, 128, 256):
        if rows <= p:
            return p
    raise AssertionError


def _plan():
    groups = []  # (n0, k, fn, l0, pad, gbase)
    rowbase = 0
    n = 0
    while n < N_BANDS:
        fn = FN[n]
        k = 1
        while n + k < N_BANDS and FN[n + k] == fn:
            k += 1
        rows = 1 + 4 * fn
        pad = _pad_for(rows)
        gbase = rowbase
        rowbase += ((k * pad + 127) // 128) * 128
        groups.append((n, k, fn, BANDS[n][0], pad, gbase))
        n += k
    nrows = rowbase  # multiple of 128
    nchunk = nrows // 128

    band_base = {}
    for (n0, k, fn, l0, pad, gbase) in groups:
        for i in range(k):
            band_base[n0 + i] = gbase + i * pad

    # matmul segments per band: (chunk, row0, klen) covering 1+4fn rows
    segs0 = []
    for n in range(N_BANDS):
        rows = 1 + 4 * FN[n]
        bb = band_base[n]
        out = []
        while rows > 0:
            ch, r0 = bb // 128, bb % 128
            kl = min(rows, 128 - r0)
            out.append((ch, r0, kl))
            bb += kl
            rows -= kl
        segs0.append(out)

    # W2 column blocks: greedy interval packing of (row0, row0+klen),
    # largest-first so full-height segments claim blocks before slivers.
    allsegs = []
    for n in range(N_BANDS):
        for si, (ch, r0, kl) in enumerate(segs0[n]):
            allsegs.append((kl, n, si, ch, r0))
    allsegs.sort(key=lambda x: -x[0])
    colblocks = []
    cb_of = {}
    for (kl, n, si, ch, r0) in allsegs:
        for cbi in range(len(colblocks) + 1):
            if cbi == len(colblocks):
                colblocks.append([])
            ivs = colblocks[cbi]
            if all(e <= r0 or s >= r0 + kl for (s, e) in ivs):
                ivs.append((r0, r0 + kl))
                cb_of[(n, si)] = cbi
                break
    ncolb = len(colblocks)
    segs = []
    for n in range(N_BANDS):
        segs.append(
            [
                (ch, r0, kl, cb_of[(n, si)])
                for si, (ch, r0, kl) in enumerate(segs0[n])
            ]
        )

    # host W2 packing map: logical row j of band n -> (w2row, colblock)
    w2map = []
    for n in range(N_BANDS):
        rows = 1 + 4 * FN[n]
        m = []
        j = 0
        for (ch, r0, kl, cb) in segs[n]:
            for q in range(kl):
                m.append((r0 + q, cb))
            j += kl
        assert j == rows
        w2map.append(m)
    return groups, nchunk, ncolb, segs, band_base, w2map


GROUPS, NCHUNK, NCOLB, SEGS, BAND_BASE, W2MAP = _plan()
NROWS = NCHUNK * 128
CHUNK_GROUP = []
for ch in range(NCHUNK):
    gi = max(i for i, g in enumerate(GROUPS) if g[5] <= ch * 128)
    CHUNK_GROUP.append(gi)

# ---------------- host-side constant prep ---------------------------------


def _prep_consts(norm_w, W, b):
    w2sb = np.zeros((128, NCOLB * D), np.float32)
    for n in range(N_BANDS):
        fn = FN[n]
        s = math.sqrt(4.0 * fn)
        row, cb = W2MAP[n][0]
        w2sb[row, cb * D:(cb + 1) * D] = b[n]
        w2rows = (s * norm_w[n][:, None] * W[n]).astype(np.float32)  # (216, 384)
        for g in range(NPLANE):
            for k in range(fn):
                row, cb = W2MAP[n][1 + g * fn + k]
                w2sb[row, cb * D:(cb + 1) * D] = w2rows[g * MAXF + k]
    idt = np.eye(128, dtype=np.float32)
    epsc = np.broadcast_to(
        (4.0 * np.asarray(FN, np.float64) * EPS).astype(np.float32)[None, :],
        (128, N_BANDS),
    ).copy()
    return w2sb, idt, epsc


# ---------------- bass kernel builder -------------------------------------

_BUILT = {}


def _build():
    if "nc" in _BUILT:
        return _BUILT["nc"]
    from contextlib import ExitStack
    import concourse.bacc as bacc
    import concourse.mybir as mybir
    from concourse import tile

    f32 = mybir.dt.float32
    f32r = mybir.dt.float32r

    nc = bacc.Bacc(None, target_bir_lowering=False)
    x_re = nc.declare_dram_parameter("X_real", [C, T, F_BINS], f32, isOutput=False)
    x_im = nc.declare_dram_parameter("X_imag", [C, T, F_BINS], f32, isOutput=False)
    w2_e = nc.declare_dram_parameter("W2", [128, NCOLB * D], f32r, isOutput=False)
    id_e = nc.declare_dram_parameter("IDT", [128, 128], f32, isOutput=False)
    eps_e = nc.declare_dram_parameter("EPSC", [128, N_BANDS], f32, isOutput=False)
    out_e = nc.declare_dram_parameter("out", [T, N_BANDS, D], f32, isOutput=True)

    GSZ = 8   # bands per output staging group
    WLD = 2   # W2 colblocks per staged load chunk

    with tile.TileContext(nc) as tc, ExitStack() as ctx:
        const = ctx.enter_context(tc.tile_pool(name="const", bufs=1))
        xtbp = ctx.enter_context(tc.tile_pool(name="xtbp", bufs=1))
        x4p = ctx.enter_context(tc.tile_pool(name="x4p", bufs=2))
        msp = ctx.enter_context(tc.tile_pool(name="msv", bufs=1))
        scr = ctx.enter_context(tc.tile_pool(name="scr", bufs=1))
        spool = ctx.enter_context(tc.tile_pool(name="stagep", bufs=2))
        trps = ctx.enter_context(tc.tile_pool(name="trp", bufs=3, space="PSUM"))
        mmps = ctx.enter_context(tc.tile_pool(name="mmp", bufs=5, space="PSUM"))

        idsb = const.tile([128, 128], f32)
        nc.sync.dma_start(out=idsb[:], in_=id_e[:])
        epsc = const.tile([128, N_BANDS], f32)
        nc.sync.dma_start(out=epsc[:], in_=eps_e[:])
        w2sb = const.tile([128, NCOLB * D], f32r)

        # double-buffered per-group gather tiles
        xcat = [[], []]
        for bi in range(2):
            for gi, (n0, k, fn, l0, pad, gbase) in enumerate(GROUPS):
                gw = (GROUPS[gi + 1][5] - gbase) if gi + 1 < len(GROUPS) else (
                    NROWS - gbase
                )
                xg = const.tile(
                    [128, gw], f32, name=f"xcat{bi}_{gi}", tag=f"xcat{bi}_{gi}"
                )
                xcat[bi].append(xg)
                nc.gpsimd.memset(xg[:], 0.0)
        xtb = [
            xtbp.tile([128, TT], f32r, name=f"xtb{m}", tag=f"xtb{m}")
            for m in range(NCHUNK)
        ]

        def phase_a(ps, gather_eng=None):
            """input DMA, band sums, sqrt, gathers + bias into xcat[ps%2]."""
            t0 = ps * TT
            xb = xcat[ps % 2]
            x4 = [
                x4p.tile([128, F_BINS], f32, tag=f"x4_{g}", name=f"x4_{g}")
                for g in range(NPLANE)
            ]
            qsum = scr.tile([128, F_BINS], f32, tag="qsum")
            sq_b = scr.tile([128, F_BINS], f32, tag="sqb")
            for g in range(NPLANE):
                xsrc = x_re if g % 2 == 0 else x_im
                nc.sync.dma_start(out=x4[g][:], in_=xsrc[g // 2, t0:t0 + TT, :])
                dst = qsum if g == 0 else sq_b
                nc.vector.tensor_tensor(
                    out=dst[:], in0=x4[g][:], in1=x4[g][:],
                    op=mybir.AluOpType.mult,
                )
                if g > 0:
                    nc.vector.tensor_tensor(
                        out=qsum[:], in0=qsum[:], in1=sq_b[:],
                        op=mybir.AluOpType.add,
                    )
            ssum_raw = msp.tile([128, N_BANDS], f32, tag="ssumr")
            for (n0, k, fn, l0, pad, gbase) in GROUPS:
                nc.vector.tensor_reduce(
                    out=ssum_raw[:, n0:n0 + k].rearrange("p (k o) -> p k o", o=1),
                    in_=qsum[:, l0:l0 + k * fn].rearrange("p (k f) -> p k f", k=k),
                    op=mybir.AluOpType.add,
                    axis=mybir.AxisListType.X,
                )
            ssum = msp.tile([128, N_BANDS], f32, tag="ssum")
            nc.vector.tensor_tensor(
                out=ssum[:], in0=ssum_raw[:], in1=epsc[:],
                op=mybir.AluOpType.add,
            )
            sqrt_n = msp.tile([128, N_BANDS], f32, tag="sqrtn")
            nc.scalar.activation(
                out=sqrt_n[:], in_=ssum[:], func=mybir.ActivationFunctionType.Sqrt
            )
            rs = msp.tile([128, N_BANDS], f32, tag=f"rs{ps}")
            nc.vector.reciprocal(rs[:], sqrt_n[:])
            geng = gather_eng or nc.gpsimd
            for gi, (n0, k, fn, l0, pad, gbase) in enumerate(GROUPS):
                xg = xb[gi]
                for g in range(NPLANE):
                    src = x4[g][:, l0:l0 + k * fn].rearrange(
                        "p (k f) -> p k f", k=k
                    )
                    dst = xg[:, 0:k * pad].rearrange(
                        "p (k q) -> p k q", k=k
                    )[:, :, 1 + g * fn:1 + (g + 1) * fn]
                    geng.tensor_copy(dst, src)
                dstb = xg[:, 0:k * pad].rearrange(
                    "p (k q) -> p k q", k=k
                )[:, :, 0:1]
                geng.tensor_copy(
                    dstb, sqrt_n[:, n0:n0 + k].rearrange("p (k o) -> p k o", o=1)
                )
            return rs

        def phase_b(ps, rs):
            """transposes, per-band matmuls, scaled eviction, output DMA."""
            t0 = ps * TT
            xb = xcat[ps % 2]
            for ch in range(NCHUNK):
                gi = CHUNK_GROUP[ch]
                off = ch * 128 - GROUPS[gi][5]
                ptr = trps.tile([128, 128], f32, tag="trp")
                nc.tensor.transpose(ptr[:], xb[gi][:, off:off + 128], idsb[:])
                if ch % 2 == 0:
                    nc.vector.tensor_copy(xtb[ch][:], ptr[:])
                else:
                    nc.scalar.copy(xtb[ch][:], ptr[:])
            for n0 in range(0, N_BANDS, GSZ):
                gn = min(GSZ, N_BANDS - n0)
                stage = spool.tile([128, GSZ * D], f32, tag="stage")
                for n in range(n0, n0 + gn):
                    pmm = mmps.tile([128, D], f32, tag="mmp")
                    nseg = len(SEGS[n])
                    for si, (ch, row0, klen, cb) in enumerate(SEGS[n]):
                        nc.tensor.matmul(
                            pmm[:],
                            lhsT=xtb[ch][row0:row0 + klen, :],
                            rhs=w2sb[row0:row0 + klen, cb * D:(cb + 1) * D],
                            start=(si == 0),
                            stop=(si == nseg - 1),
                            tile_position=(row0, 0),
                        )
                    slot = stage[:, (n - n0) * D:(n - n0 + 1) * D]
                    if n % 2 == 0:
                        nc.vector.tensor_scalar_mul(slot, pmm[:], rs[:, n:n + 1])
                    else:
                        nc.scalar.mul(slot, pmm[:], rs[:, n:n + 1])
                nc.sync.dma_start(
                    out=out_e[t0:t0 + TT, n0:n0 + gn, :],
                    in_=stage[:, 0:gn * D].rearrange("p (n d) -> p n d", n=gn),
                )

        def load_w2():
            nc.sync.dma_start(out=w2sb[:], in_=w2_e[:])

        # software pipeline: A(0) w2 A(1) B(0) A(2) B(1) A(3) B(2) B(3)
        rs_of = {}
        rs_of[0] = phase_a(0, gather_eng=nc.vector)
        load_w2()
        rs_of[1] = phase_a(1)
        phase_b(0, rs_of[0])
        rs_of[2] = phase_a(2)
        phase_b(1, rs_of[1])
        rs_of[3] = phase_a(3)
        phase_b(2, rs_of[2])
        phase_b(3, rs_of[3])

    nc.finalize()
    _BUILT["nc"] = nc
    return nc


# ---------------- entry points --------------------------------------------


def _run(in_maps, trace=False):
    from concourse.bass_utils import run_bass_kernel_spmd

    nc = _build()
    return run_bass_kernel_spmd(nc, in_maps, core_ids=list(range(8)), trace=trace)


def _run_traced(in_maps, tmpdir=None):
    from concourse.bass_utils import run_bass_kernel_spmd

    nc = _build()
    return run_bass_kernel_spmd(
        nc, in_maps, core_ids=list(range(8)), trace=True, tmpdir=tmpdir
    )


def _make_in_maps(X_real, X_imag, norm_w, W, b):
    X_real = np.ascontiguousarray(np.asarray(X_real, np.float32))
    X_imag = np.ascontiguousarray(np.asarray(X_imag, np.float32))
    w2sb, idt, epsc = _prep_consts(
        np.asarray(norm_w, np.float32), np.asarray(W, np.float32),
        np.asarray(b, np.float32),
    )
    return [
        {
            "X_real": X_real[i],
            "X_imag": X_imag[i],
            "W2": w2sb,
            "IDT": idt,
            "EPSC": epsc,
        }
        for i in range(B)
    ]


def kernel(X_real, X_imag, norm_w, W, b):
    res = _run(_make_in_maps(X_real, X_imag, norm_w, W, b), trace=False)
    return np.stack([res.results[i]["out"] for i in range(B)]).astype(np.float32)


def kernel_profiled(X_real, X_imag, norm_w, W, b):
    res = _run(_make_in_maps(X_real, X_imag, norm_w, W, b), trace=True)
    out = np.stack([res.results[i]["out"] for i in range(B)]).astype(np.float32)
    return out, res


if __name__ == "__main__":
    print(f"NCHUNK={NCHUNK} NCOLB={NCOLB} NROWS={NROWS}")
    print(f"groups: {[(g[0], g[1], g[2], g[4]) for g in GROUPS]}")
    print(f"matmul segs per pass: {sum(len(s) for s in SEGS)}")
    per_part = (2 * NROWS * 4 + NCHUNK * TT * 4 + NCOLB * D * 4
                + 2 * NPLANE * F_BINS * 4 + 2 * 4 * D * 4 + 3 * 4 * D * 4) / 1024
    print(f"approx SBUF per partition: {per_part:.0f} KB")


# revision 28
# speedup vs baseline: 1.0892x; 1.0280x over previous
"""BandSplit Trainium2 kernel: 8-core data-parallel over batch.

out[b,t,n,d] = rsqrt(ms + eps) * (x_band @ (norm_w * W)) + bias
with ms = sum(x_band^2)/(4*fn),  x_band = contiguous freq slices of X.

Math restructure (exact):
  rsqrt(ms + eps) = sqrt(4fn) / sqrt(ssum),  ssum = sum(x^2) + 4*fn*eps
  out = (1/sqrt(ssum)) * (x @ W2 + sqrt(ssum) * b)     [bias-row trick]
  where W2[n,p,:] = sqrt(4fn) * norm_w[n,p] * W[n,p,:]

Per core (one batch element, T=512 tokens, 4 passes of 128):
  1. DMA X planes into natural layout (t part, f free).
  2. ssum per band via one fused multiply-reduce per band (eps as initial).
  3. Free-axis gather (on GpSimd) rearranges columns into the packed
     band-major row order; bands grouped by equal width give affine 3D
     APs, one copy per (plane, width-group). Bias slots get sqrt(ssum).
  4. PE-transpose each 128-column block -> packed row chunks (XtB, f32r).
  5. Per band: 1-2 float32r matmuls (K=4fn+1, M=128 tokens, N=384).
  6. Evict PSUM->SBUF scaled by 1/sqrt(ssum) per token; DMA out.
"""

import math
import numpy as np

# ---------------- problem geometry (hardcoded, matches reference) ----------
SR, N_FFT, D = 44100, 2048, 384
RANGES = [(1000, 2), (2000, 4), (4000, 12), (8000, 24), (16000, 48)]


def _compute_bands(sr=SR, n_fft=N_FFT):
    hz_per_bin = sr / n_fft
    max_bin = n_fft // 2 + 1
    boundaries = [0]
    for hi_hz, bins in RANGES:
        hi_bin = math.floor(hi_hz / hz_per_bin)
        while boundaries[-1] + bins <= hi_bin and boundaries[-1] + bins <= max_bin:
            boundaries.append(boundaries[-1] + bins)
    if boundaries[-1] < max_bin:
        remaining = max_bin - boundaries[-1]
        step = math.ceil(remaining / 6)
        b = boundaries[-1]
        while b + step < max_bin:
            b += step
            boundaries.append(b)
        boundaries.append(max_bin)
    return [(boundaries[i], boundaries[i + 1]) for i in range(len(boundaries) - 1)]


BANDS = _compute_bands()
N_BANDS = len(BANDS)
assert N_BANDS == 62
FN = [r - l for l, r in BANDS]
MAXF = max(FN)
F_BINS = N_FFT // 2 + 1  # 1025
EPS = 1e-8
B, C, T = 8, 2, 512
TT = 128  # tokens per pass (matmul M)
NP = T // TT  # 4 passes
NPLANE = 4  # (c, ri) combinations, g = 2*c + ri

# ---------------- regular row layout by equal-width band groups ------------
# logical row j of band n: j=0 -> bias; j=1+g*fn+k -> plane g, freq l+k.
# Bands with equal fn are consecutive; within a group each band's rows
# start at G_base + i*pad, giving affine gather patterns.


def _pad_for(rows):
    for p in (32, 64, 128, 256):
        if rows <= p:
            return p
    raise AssertionError


def _plan():
    groups = []  # (n0, k, fn, l0, pad, gbase)
    rowbase = 0
    n = 0
    while n < N_BANDS:
        fn = FN[n]
        k = 1
        while n + k < N_BANDS and FN[n + k] == fn:
            k += 1
        rows = 1 + 4 * fn
        pad = _pad_for(rows)
        gbase = rowbase
        rowbase += ((k * pad + 127) // 128) * 128
        groups.append((n, k, fn, BANDS[n][0], pad, gbase))
        n += k
    nrows = rowbase  # multiple of 128
    nchunk = nrows // 128

    band_base = {}
    for (n0, k, fn, l0, pad, gbase) in groups:
        for i in range(k):
            band_base[n0 + i] = gbase + i * pad

    # matmul segments per band: (chunk, row0, klen) covering 1+4fn rows
    segs0 = []
    for n in range(N_BANDS):
        rows = 1 + 4 * FN[n]
        bb = band_base[n]
        out = []
        while rows > 0:
            ch, r0 = bb // 128, bb % 128
            kl = min(rows, 128 - r0)
            out.append((ch, r0, kl))
            bb += kl
            rows -= kl
        segs0.append(out)

    # W2 column blocks: greedy interval packing of (row0, row0+klen),
    # largest-first so full-height segments claim blocks before slivers.
    allsegs = []
    for n in range(N_BANDS):
        for si, (ch, r0, kl) in enumerate(segs0[n]):
            allsegs.append((kl, n, si, ch, r0))
    allsegs.sort(key=lambda x: -x[0])
    colblocks = []
    cb_of = {}
    for (kl, n, si, ch, r0) in allsegs:
        for cbi in range(len(colblocks) + 1):
            if cbi == len(colblocks):
                colblocks.append([])
            ivs = colblocks[cbi]
            if all(e <= r0 or s >= r0 + kl for (s, e) in ivs):
                ivs.append((r0, r0 + kl))
                cb_of[(n, si)] = cbi
                break
    ncolb = len(colblocks)
    segs = []
    for n in range(N_BANDS):
        segs.append(
            [
                (ch, r0, kl, cb_of[(n, si)])
                for si, (ch, r0, kl) in enumerate(segs0[n])
            ]
        )

    # host W2 packing map: logical row j of band n -> (w2row, colblock)
    w2map = []
    for n in range(N_BANDS):
        rows = 1 + 4 * FN[n]
        m = []
        j = 0
        for (ch, r0, kl, cb) in segs[n]:
            for q in range(kl):
                m.append((r0 + q, cb))
            j += kl
        assert j == rows
        w2map.append(m)
    return groups, nchunk, ncolb, segs, band_base, w2map


GROUPS, NCHUNK, NCOLB, SEGS, BAND_BASE, W2MAP = _plan()
NROWS = NCHUNK * 128
CHUNK_GROUP = []
for ch in range(NCHUNK):
    gi = max(i for i, g in enumerate(GROUPS) if g[5] <= ch * 128)
    CHUNK_GROUP.append(gi)

# ---------------- host-side constant prep ---------------------------------


def _prep_consts(norm_w, W, b):
    w2sb = np.zeros((128, NCOLB * D), np.float32)
    for n in range(N_BANDS):
        fn = FN[n]
        s = math.sqrt(4.0 * fn)
        row, cb = W2MAP[n][0]
        w2sb[row, cb * D:(cb + 1) * D] = b[n]
        w2rows = (s * norm_w[n][:, None] * W[n]).astype(np.float32)  # (216, 384)
        for g in range(NPLANE):
            for k in range(fn):
                row, cb = W2MAP[n][1 + g * fn + k]
                w2sb[row, cb * D:(cb + 1) * D] = w2rows[g * MAXF + k]
    idt = np.eye(128, dtype=np.float32)
    epsc = np.broadcast_to(
        (4.0 * np.asarray(FN, np.float64) * EPS).astype(np.float32)[None, :],
        (128, N_BANDS),
    ).copy()
    return w2sb, idt, epsc


# ---------------- bass kernel builder -------------------------------------

_BUILT = {}


def _build():
    if "nc" in _BUILT:
        return _BUILT["nc"]
    from contextlib import ExitStack
    import concourse.bacc as bacc
    import concourse.mybir as mybir
    from concourse import tile

    f32 = mybir.dt.float32
    f32r = mybir.dt.float32r

    nc = bacc.Bacc(None, target_bir_lowering=False)
    x_re = nc.declare_dram_parameter("X_real", [C, T, F_BINS], f32, isOutput=False)
    x_im = nc.declare_dram_parameter("X_imag", [C, T, F_BINS], f32, isOutput=False)
    w2_e = nc.declare_dram_parameter("W2", [128, NCOLB * D], f32r, isOutput=False)
    id_e = nc.declare_dram_parameter("IDT", [128, 128], f32, isOutput=False)
    eps_e = nc.declare_dram_parameter("EPSC", [128, N_BANDS], f32, isOutput=False)
    out_e = nc.declare_dram_parameter("out", [T, N_BANDS, D], f32, isOutput=True)

    GSZ = 8   # bands per output staging group
    WLD = 2   # W2 colblocks per staged load chunk

    with tile.TileContext(nc) as tc, ExitStack() as ctx:
        const = ctx.enter_context(tc.tile_pool(name="const", bufs=1))
        xtbp = ctx.enter_context(tc.tile_pool(name="xtbp", bufs=1))
        x4p = ctx.enter_context(tc.tile_pool(name="x4p", bufs=2))
        msp = ctx.enter_context(tc.tile_pool(name="msv", bufs=1))
        scr = ctx.enter_context(tc.tile_pool(name="scr", bufs=1))
        spool = ctx.enter_context(tc.tile_pool(name="stagep", bufs=2))
        trps = ctx.enter_context(tc.tile_pool(name="trp", bufs=3, space="PSUM"))
        mmps = ctx.enter_context(tc.tile_pool(name="mmp", bufs=5, space="PSUM"))

        idsb = const.tile([128, 128], f32)
        nc.sync.dma_start(out=idsb[:], in_=id_e[:])
        epsc = const.tile([128, N_BANDS], f32)
        nc.sync.dma_start(out=epsc[:], in_=eps_e[:])
        w2sb = const.tile([128, NCOLB * D], f32r)

        # double-buffered per-group gather tiles
        xcat = [[], []]
        for bi in range(2):
            for gi, (n0, k, fn, l0, pad, gbase) in enumerate(GROUPS):
                gw = (GROUPS[gi + 1][5] - gbase) if gi + 1 < len(GROUPS) else (
                    NROWS - gbase
                )
                xg = const.tile(
                    [128, gw], f32, name=f"xcat{bi}_{gi}", tag=f"xcat{bi}_{gi}"
                )
                xcat[bi].append(xg)
                nc.gpsimd.memset(xg[:], 0.0)
        xtb = [
            xtbp.tile([128, TT], f32r, name=f"xtb{m}", tag=f"xtb{m}")
            for m in range(NCHUNK)
        ]

        def phase_a(ps, gather_eng=None):
            """input DMA; per-group: band sums, sqrt, gathers + bias."""
            t0 = ps * TT
            xb = xcat[ps % 2]
            geng = gather_eng or nc.gpsimd
            x4 = [
                x4p.tile([128, F_BINS], f32, tag=f"x4_{g}", name=f"x4_{g}")
                for g in range(NPLANE)
            ]
            for g in range(NPLANE):
                xsrc = x_re if g % 2 == 0 else x_im
                nc.sync.dma_start(out=x4[g][:], in_=xsrc[g // 2, t0:t0 + TT, :])
            rs = msp.tile([128, N_BANDS], f32, tag=f"rs{ps}", name=f"rs{ps}")
            for gi, (n0, k, fn, l0, pad, gbase) in enumerate(GROUPS):
                kfn = k * fn
                qg = scr.tile([128, kfn], f32, tag=f"q{gi}", name=f"q{gi}")
                sb = scr.tile([128, kfn], f32, tag=f"sb{gi}", name=f"sb{gi}")
                for g in range(NPLANE):
                    dst = qg if g == 0 else sb
                    nc.vector.tensor_tensor(
                        out=dst[:], in0=x4[g][:, l0:l0 + kfn],
                        in1=x4[g][:, l0:l0 + kfn], op=mybir.AluOpType.mult,
                    )
                    if g > 0:
                        nc.vector.tensor_tensor(
                            out=qg[:], in0=qg[:], in1=sb[:],
                            op=mybir.AluOpType.add,
                        )
                ssr = scr.tile([128, k], f32, tag=f"ssr{gi}", name=f"ssr{gi}")
                nc.vector.tensor_reduce(
                    out=ssr[:].rearrange("p (k o) -> p k o", o=1),
                    in_=qg[:].rearrange("p (k f) -> p k f", k=k),
                    op=mybir.AluOpType.add,
                    axis=mybir.AxisListType.X,
                )
                sqg = scr.tile([128, k], f32, tag=f"sqg{gi}", name=f"sqg{gi}")
                nc.vector.tensor_tensor(
                    out=ssr[:], in0=ssr[:], in1=epsc[:, n0:n0 + k],
                    op=mybir.AluOpType.add,
                )
                nc.scalar.activation(
                    out=sqg[:], in_=ssr[:],
                    func=mybir.ActivationFunctionType.Sqrt,
                )
                nc.vector.reciprocal(rs[:, n0:n0 + k], sqg[:])
                xg = xb[gi]
                for g in range(NPLANE):
                    src = x4[g][:, l0:l0 + kfn].rearrange("p (k f) -> p k f", k=k)
                    dst = xg[:, 0:k * pad].rearrange(
                        "p (k q) -> p k q", k=k
                    )[:, :, 1 + g * fn:1 + (g + 1) * fn]
                    geng.tensor_copy(dst, src)
                dstb = xg[:, 0:k * pad].rearrange(
                    "p (k q) -> p k q", k=k
                )[:, :, 0:1]
                geng.tensor_copy(
                    dstb, sqg[:].rearrange("p (k o) -> p k o", o=1)
                )
            return rs

        def phase_b(ps, rs):
            """transposes, per-band matmuls, scaled eviction, output DMA."""
            t0 = ps * TT
            xb = xcat[ps % 2]
            for ch in range(NCHUNK):
                gi = CHUNK_GROUP[ch]
                off = ch * 128 - GROUPS[gi][5]
                ptr = trps.tile([128, 128], f32, tag="trp")
                nc.tensor.transpose(ptr[:], xb[gi][:, off:off + 128], idsb[:])
                if ch % 2 == 0:
                    nc.vector.tensor_copy(xtb[ch][:], ptr[:])
                else:
                    nc.scalar.copy(xtb[ch][:], ptr[:])
            for n0 in range(0, N_BANDS, GSZ):
                gn = min(GSZ, N_BANDS - n0)
                stage = spool.tile([128, GSZ * D], f32, tag="stage")
                for n in range(n0, n0 + gn):
                    pmm = mmps.tile([128, D], f32, tag="mmp")
                    nseg = len(SEGS[n])
                    for si, (ch, row0, klen, cb) in enumerate(SEGS[n]):
                        nc.tensor.matmul(
                            pmm[:],
                            lhsT=xtb[ch][row0:row0 + klen, :],
                            rhs=w2sb[row0:row0 + klen, cb * D:(cb + 1) * D],
                            start=(si == 0),
                            stop=(si == nseg - 1),
                            tile_position=(row0, 0),
                        )
                    slot = stage[:, (n - n0) * D:(n - n0 + 1) * D]
                    if n % 2 == 0:
                        nc.vector.tensor_scalar_mul(slot, pmm[:], rs[:, n:n + 1])
                    else:
                        nc.scalar.mul(slot, pmm[:], rs[:, n:n + 1])
                nc.sync.dma_start(
                    out=out_e[t0:t0 + TT, n0:n0 + gn, :],
                    in_=stage[:, 0:gn * D].rearrange("p (n d) -> p n d", n=gn),
                )

        def load_w2():
            nc.sync.dma_start(out=w2sb[:], in_=w2_e[:])

        # software pipeline: A(0) w2 A(1) B(0) A(2) B(1) A(3) B(2) B(3)
        rs_of = {}
        rs_of[0] = phase_a(0, gather_eng=nc.vector)
        load_w2()
        rs_of[1] = phase_a(1)
        phase_b(0, rs_of[0])
        rs_of[2] = phase_a(2)
        phase_b(1, rs_of[1])
        rs_of[3] = phase_a(3)
        phase_b(2, rs_of[2])
        phase_b(3, rs_of[3])

    nc.finalize()
    _BUILT["nc"] = nc
    return nc


# ---------------- entry points --------------------------------------------


def _run(in_maps, trace=False):
    from concourse.bass_utils import run_bass_kernel_spmd

    nc = _build()
    return run_bass_kernel_spmd(nc, in_maps, core_ids=list(range(8)), trace=trace)


def _run_traced(in_maps, tmpdir=None):
    from concourse.bass_utils import run_bass_kernel_spmd

    nc = _build()
    return run_bass_kernel_spmd(
        nc, in_maps, core_ids=list(range(8)), trace=True, tmpdir=tmpdir
    )


def _make_in_maps(X_real, X_imag, norm_w, W, b):
    X_real = np.ascontiguousarray(np.asarray(X_real, np.float32))
    X_imag = np.ascontiguousarray(np.asarray(X_imag, np.float32))
    w2sb, idt, epsc = _prep_consts(
        np.asarray(norm_w, np.float32), np.asarray(W, np.float32),
        np.asarray(b, np.float32),
    )
    return [
        {
            "X_real": X_real[i],
            "X_imag": X_imag[i],
            "W2": w2sb,
            "IDT": idt,
            "EPSC": epsc,
        }
        for i in range(B)
    ]


def kernel(X_real, X_imag, norm_w, W, b):
    res = _run(_make_in_maps(X_real, X_imag, norm_w, W, b), trace=False)
    return np.stack([res.results[i]["out"] for i in range(B)]).astype(np.float32)


def kernel_profiled(X_real, X_imag, norm_w, W, b):
    res = _run(_make_in_maps(X_real, X_imag, norm_w, W, b), trace=True)
    out = np.stack([res.results[i]["out"] for i in range(B)]).astype(np.float32)
    return out, res


if __name__ == "__main__":
    print(f"NCHUNK={NCHUNK} NCOLB={NCOLB} NROWS={NROWS}")
    print(f"groups: {[(g[0], g[1], g[2], g[4]) for g in GROUPS]}")
    print(f"matmul segs per pass: {sum(len(s) for s in SEGS)}")
    per_part = (2 * NROWS * 4 + NCHUNK * TT * 4 + NCOLB * D * 4
                + 2 * NPLANE * F_BINS * 4 + 2 * 4 * D * 4 + 3 * 4 * D * 4) / 1024
    print(f"approx SBUF per partition: {per_part:.0f} KB")


# revision 29
# speedup vs baseline: 1.1837x; 1.0868x over previous
"""BandSplit Trainium2 kernel: 8-core data-parallel over batch.

out[b,t,n,d] = rsqrt(ms + eps) * (x_band @ (norm_w * W)) + bias
with ms = sum(x_band^2)/(4*fn),  x_band = contiguous freq slices of X.

Math restructure (exact):
  rsqrt(ms + eps) = sqrt(4fn) / sqrt(ssum),  ssum = sum(x^2) + 4*fn*eps
  out = (1/sqrt(ssum)) * (x @ W2 + sqrt(ssum) * b)     [bias-row trick]
  where W2[n,p,:] = sqrt(4fn) * norm_w[n,p] * W[n,p,:]

Per core (one batch element, T=512 tokens, 4 passes of 128):
  1. DMA X planes into natural layout (t part, f free).
  2. ssum per band via one fused multiply-reduce per band (eps as initial).
  3. Free-axis gather (on GpSimd) rearranges columns into the packed
     band-major row order; bands grouped by equal width give affine 3D
     APs, one copy per (plane, width-group). Bias slots get sqrt(ssum).
  4. PE-transpose each 128-column block -> packed row chunks (XtB, f32r).
  5. Per band: 1-2 float32r matmuls (K=4fn+1, M=128 tokens, N=384).
  6. Evict PSUM->SBUF scaled by 1/sqrt(ssum) per token; DMA out.
"""

import math
import numpy as np

# ---------------- problem geometry (hardcoded, matches reference) ----------
SR, N_FFT, D = 44100, 2048, 384
RANGES = [(1000, 2), (2000, 4), (4000, 12), (8000, 24), (16000, 48)]


def _compute_bands(sr=SR, n_fft=N_FFT):
    hz_per_bin = sr / n_fft
    max_bin = n_fft // 2 + 1
    boundaries = [0]
    for hi_hz, bins in RANGES:
        hi_bin = math.floor(hi_hz / hz_per_bin)
        while boundaries[-1] + bins <= hi_bin and boundaries[-1] + bins <= max_bin:
            boundaries.append(boundaries[-1] + bins)
    if boundaries[-1] < max_bin:
        remaining = max_bin - boundaries[-1]
        step = math.ceil(remaining / 6)
        b = boundaries[-1]
        while b + step < max_bin:
            b += step
            boundaries.append(b)
        boundaries.append(max_bin)
    return [(boundaries[i], boundaries[i + 1]) for i in range(len(boundaries) - 1)]


BANDS = _compute_bands()
N_BANDS = len(BANDS)
assert N_BANDS == 62
FN = [r - l for l, r in BANDS]
MAXF = max(FN)
F_BINS = N_FFT // 2 + 1  # 1025
EPS = 1e-8
B, C, T = 8, 2, 512
TT = 128  # tokens per pass (matmul M)
NP = T // TT  # 4 passes
NPLANE = 4  # (c, ri) combinations, g = 2*c + ri

# ---------------- regular row layout by equal-width band groups ------------
# logical row j of band n: j=0 -> bias; j=1+g*fn+k -> plane g, freq l+k.
# Bands with equal fn are consecutive; within a group each band's rows
# start at G_base + i*pad, giving affine gather patterns.


def _pad_for(rows):
    for p in (32, 64, 128, 256):
        if rows <= p:
            return p
    raise AssertionError


def _plan():
    groups = []  # (n0, k, fn, l0, pad, gbase)
    rowbase = 0
    n = 0
    while n < N_BANDS:
        fn = FN[n]
        k = 1
        while n + k < N_BANDS and FN[n + k] == fn:
            k += 1
        rows = 1 + 4 * fn
        pad = _pad_for(rows)
        gbase = rowbase
        rowbase += ((k * pad + 127) // 128) * 128
        groups.append((n, k, fn, BANDS[n][0], pad, gbase))
        n += k
    nrows = rowbase  # multiple of 128
    nchunk = nrows // 128

    band_base = {}
    for (n0, k, fn, l0, pad, gbase) in groups:
        for i in range(k):
            band_base[n0 + i] = gbase + i * pad

    # matmul segments per band: (chunk, row0, klen) covering 1+4fn rows
    segs0 = []
    for n in range(N_BANDS):
        rows = 1 + 4 * FN[n]
        bb = band_base[n]
        out = []
        while rows > 0:
            ch, r0 = bb // 128, bb % 128
            kl = min(rows, 128 - r0)
            out.append((ch, r0, kl))
            bb += kl
            rows -= kl
        segs0.append(out)

    # W2 column blocks: greedy interval packing of (row0, row0+klen),
    # largest-first so full-height segments claim blocks before slivers.
    allsegs = []
    for n in range(N_BANDS):
        for si, (ch, r0, kl) in enumerate(segs0[n]):
            allsegs.append((kl, n, si, ch, r0))
    allsegs.sort(key=lambda x: -x[0])
    colblocks = []
    cb_of = {}
    for (kl, n, si, ch, r0) in allsegs:
        for cbi in range(len(colblocks) + 1):
            if cbi == len(colblocks):
                colblocks.append([])
            ivs = colblocks[cbi]
            if all(e <= r0 or s >= r0 + kl for (s, e) in ivs):
                ivs.append((r0, r0 + kl))
                cb_of[(n, si)] = cbi
                break
    ncolb = len(colblocks)
    segs = []
    for n in range(N_BANDS):
        segs.append(
            [
                (ch, r0, kl, cb_of[(n, si)])
                for si, (ch, r0, kl) in enumerate(segs0[n])
            ]
        )

    # host W2 packing map: logical row j of band n -> (w2row, colblock)
    w2map = []
    for n in range(N_BANDS):
        rows = 1 + 4 * FN[n]
        m = []
        j = 0
        for (ch, r0, kl, cb) in segs[n]:
            for q in range(kl):
                m.append((r0 + q, cb))
            j += kl
        assert j == rows
        w2map.append(m)
    return groups, nchunk, ncolb, segs, band_base, w2map


GROUPS, NCHUNK, NCOLB, SEGS, BAND_BASE, W2MAP = _plan()
NROWS = NCHUNK * 128
CHUNK_GROUP = []
for ch in range(NCHUNK):
    gi = max(i for i, g in enumerate(GROUPS) if g[5] <= ch * 128)
    CHUNK_GROUP.append(gi)

# ---------------- host-side constant prep ---------------------------------


def _prep_consts(norm_w, W, b):
    import ml_dtypes
    w2sb = np.zeros((128, NCOLB * D), np.float32)
    for n in range(N_BANDS):
        fn = FN[n]
        s = math.sqrt(4.0 * fn)
        row, cb = W2MAP[n][0]
        w2sb[row, cb * D:(cb + 1) * D] = b[n]
        w2rows = (s * norm_w[n][:, None] * W[n]).astype(np.float32)  # (216, 384)
        for g in range(NPLANE):
            for k in range(fn):
                row, cb = W2MAP[n][1 + g * fn + k]
                w2sb[row, cb * D:(cb + 1) * D] = w2rows[g * MAXF + k]
    w2sb = w2sb.astype(ml_dtypes.bfloat16)
    idt = np.eye(128, dtype=np.float32)
    epsc = np.broadcast_to(
        (4.0 * np.asarray(FN, np.float64) * EPS).astype(np.float32)[None, :],
        (128, N_BANDS),
    ).copy()
    return w2sb, idt, epsc


# ---------------- bass kernel builder -------------------------------------

_BUILT = {}


def _build():
    if "nc" in _BUILT:
        return _BUILT["nc"]
    from contextlib import ExitStack
    import concourse.bacc as bacc
    import concourse.mybir as mybir
    from concourse import tile

    f32 = mybir.dt.float32
    f32r = mybir.dt.float32r
    mmdt = mybir.dt.bfloat16

    nc = bacc.Bacc(None, target_bir_lowering=False)
    x_re = nc.declare_dram_parameter("X_real", [C, T, F_BINS], f32, isOutput=False)
    x_im = nc.declare_dram_parameter("X_imag", [C, T, F_BINS], f32, isOutput=False)
    w2_e = nc.declare_dram_parameter("W2", [128, NCOLB * D], mmdt, isOutput=False)
    id_e = nc.declare_dram_parameter("IDT", [128, 128], f32, isOutput=False)
    eps_e = nc.declare_dram_parameter("EPSC", [128, N_BANDS], f32, isOutput=False)
    out_e = nc.declare_dram_parameter("out", [T, N_BANDS, D], f32, isOutput=True)

    GSZ = 8   # bands per output staging group
    WLD = 2   # W2 colblocks per staged load chunk

    with tile.TileContext(nc) as tc, ExitStack() as ctx:
        const = ctx.enter_context(tc.tile_pool(name="const", bufs=1))
        xtbp = ctx.enter_context(tc.tile_pool(name="xtbp", bufs=1))
        x4p = ctx.enter_context(tc.tile_pool(name="x4p", bufs=2))
        msp = ctx.enter_context(tc.tile_pool(name="msv", bufs=1))
        scr = ctx.enter_context(tc.tile_pool(name="scr", bufs=2))
        spool = ctx.enter_context(tc.tile_pool(name="stagep", bufs=3))
        trps = ctx.enter_context(tc.tile_pool(name="trp", bufs=3, space="PSUM"))
        mmps = ctx.enter_context(tc.tile_pool(name="mmp", bufs=5, space="PSUM"))

        idsb = const.tile([128, 128], f32)
        nc.sync.dma_start(out=idsb[:], in_=id_e[:])
        epsc = const.tile([128, N_BANDS], f32)
        nc.sync.dma_start(out=epsc[:], in_=eps_e[:])
        w2sb = const.tile([128, NCOLB * D], mmdt)

        # double-buffered per-group gather tiles
        xcat = [[], []]
        for bi in range(2):
            for gi, (n0, k, fn, l0, pad, gbase) in enumerate(GROUPS):
                gw = (GROUPS[gi + 1][5] - gbase) if gi + 1 < len(GROUPS) else (
                    NROWS - gbase
                )
                xg = const.tile(
                    [128, gw], f32, name=f"xcat{bi}_{gi}", tag=f"xcat{bi}_{gi}"
                )
                xcat[bi].append(xg)
                nc.gpsimd.memset(xg[:], 0.0)
        xtb = [
            xtbp.tile([128, TT], mmdt, name=f"xtb{m}", tag=f"xtb{m}")
            for m in range(NCHUNK)
        ]

        def phase_a(ps, gather_eng=None):
            """input DMA; per-group: band sums, sqrt, gathers + bias."""
            t0 = ps * TT
            xb = xcat[ps % 2]
            geng = gather_eng or nc.gpsimd
            x4 = [
                x4p.tile([128, F_BINS], f32, tag=f"x4_{g}", name=f"x4_{g}")
                for g in range(NPLANE)
            ]
            for g in range(NPLANE):
                xsrc = x_re if g % 2 == 0 else x_im
                nc.sync.dma_start(out=x4[g][:], in_=xsrc[g // 2, t0:t0 + TT, :])
            rs = msp.tile([128, N_BANDS], f32, tag=f"rs{ps}", name=f"rs{ps}")
            for gi, (n0, k, fn, l0, pad, gbase) in enumerate(GROUPS):
                kfn = k * fn
                qg = scr.tile([128, kfn], f32, tag=f"q{gi}", name=f"q{gi}")
                sb = scr.tile([128, kfn], f32, tag=f"sb{gi}", name=f"sb{gi}")
                for g in range(NPLANE):
                    dst = qg if g == 0 else sb
                    nc.vector.tensor_tensor(
                        out=dst[:], in0=x4[g][:, l0:l0 + kfn],
                        in1=x4[g][:, l0:l0 + kfn], op=mybir.AluOpType.mult,
                    )
                    if g > 0:
                        nc.vector.tensor_tensor(
                            out=qg[:], in0=qg[:], in1=sb[:],
                            op=mybir.AluOpType.add,
                        )
                ssr = scr.tile([128, k], f32, tag=f"ssr{gi}", name=f"ssr{gi}")
                nc.vector.tensor_reduce(
                    out=ssr[:].rearrange("p (k o) -> p k o", o=1),
                    in_=qg[:].rearrange("p (k f) -> p k f", k=k),
                    op=mybir.AluOpType.add,
                    axis=mybir.AxisListType.X,
                )
                sqg = scr.tile([128, k], f32, tag=f"sqg{gi}", name=f"sqg{gi}")
                nc.vector.tensor_tensor(
                    out=ssr[:], in0=ssr[:], in1=epsc[:, n0:n0 + k],
                    op=mybir.AluOpType.add,
                )
                nc.scalar.activation(
                    out=sqg[:], in_=ssr[:],
                    func=mybir.ActivationFunctionType.Sqrt,
                )
                nc.vector.reciprocal(rs[:, n0:n0 + k], sqg[:])
                xg = xb[gi]
                for g in range(NPLANE):
                    src = x4[g][:, l0:l0 + kfn].rearrange("p (k f) -> p k f", k=k)
                    dst = xg[:, 0:k * pad].rearrange(
                        "p (k q) -> p k q", k=k
                    )[:, :, 1 + g * fn:1 + (g + 1) * fn]
                    geng.tensor_copy(dst, src)
                dstb = xg[:, 0:k * pad].rearrange(
                    "p (k q) -> p k q", k=k
                )[:, :, 0:1]
                geng.tensor_copy(
                    dstb, sqg[:].rearrange("p (k o) -> p k o", o=1)
                )
            return rs

        def phase_b(ps, rs):
            """transposes, per-band matmuls, scaled eviction, output DMA."""
            t0 = ps * TT
            xb = xcat[ps % 2]
            for ch in range(NCHUNK):
                gi = CHUNK_GROUP[ch]
                off = ch * 128 - GROUPS[gi][5]
                ptr = trps.tile([128, 128], f32, tag="trp")
                nc.tensor.transpose(ptr[:], xb[gi][:, off:off + 128], idsb[:])
                if ch % 2 == 0:
                    nc.vector.tensor_copy(xtb[ch][:], ptr[:])
                else:
                    nc.scalar.copy(xtb[ch][:], ptr[:])
            for n0 in range(0, N_BANDS, GSZ):
                gn = min(GSZ, N_BANDS - n0)
                stage = spool.tile([128, GSZ * D], f32, tag="stage")
                for n in range(n0, n0 + gn):
                    pmm = mmps.tile([128, D], f32, tag="mmp")
                    nseg = len(SEGS[n])
                    for si, (ch, row0, klen, cb) in enumerate(SEGS[n]):
                        nc.tensor.matmul(
                            pmm[:],
                            lhsT=xtb[ch][row0:row0 + klen, :],
                            rhs=w2sb[row0:row0 + klen, cb * D:(cb + 1) * D],
                            start=(si == 0),
                            stop=(si == nseg - 1),
                            tile_position=(row0, 0),
                        )
                    slot = stage[:, (n - n0) * D:(n - n0 + 1) * D]
                    if n % 2 == 0:
                        nc.vector.tensor_scalar_mul(slot, pmm[:], rs[:, n:n + 1])
                    else:
                        nc.scalar.mul(slot, pmm[:], rs[:, n:n + 1])
                nc.sync.dma_start(
                    out=out_e[t0:t0 + TT, n0:n0 + gn, :],
                    in_=stage[:, 0:gn * D].rearrange("p (n d) -> p n d", n=gn),
                )

        def load_w2():
            nc.sync.dma_start(out=w2sb[:], in_=w2_e[:])

        # software pipeline: A(0) w2 A(1) B(0) A(2) B(1) A(3) B(2) B(3)
        rs_of = {}
        rs_of[0] = phase_a(0, gather_eng=nc.vector)
        load_w2()
        rs_of[1] = phase_a(1)
        phase_b(0, rs_of[0])
        rs_of[2] = phase_a(2)
        phase_b(1, rs_of[1])
        rs_of[3] = phase_a(3)
        phase_b(2, rs_of[2])
        phase_b(3, rs_of[3])

    nc.finalize()
    _BUILT["nc"] = nc
    return nc


# ---------------- entry points --------------------------------------------


def _run(in_maps, trace=False):
    from concourse.bass_utils import run_bass_kernel_spmd

    nc = _build()
    return run_bass_kernel_spmd(nc, in_maps, core_ids=list(range(8)), trace=trace)


def _run_traced(in_maps, tmpdir=None):
    from concourse.bass_utils import run_bass_kernel_spmd

    nc = _build()
    return run_bass_kernel_spmd(
        nc, in_maps, core_ids=list(range(8)), trace=True, tmpdir=tmpdir
    )


def _make_in_maps(X_real, X_imag, norm_w, W, b):
    X_real = np.ascontiguousarray(np.asarray(X_real, np.float32))
    X_imag = np.ascontiguousarray(np.asarray(X_imag, np.float32))
    w2sb, idt, epsc = _prep_consts(
        np.asarray(norm_w, np.float32), np.asarray(W, np.float32),
        np.asarray(b, np.float32),
    )
    return [
        {
            "X_real": X_real[i],
            "X_imag": X_imag[i],
            "W2": w2sb,
            "IDT": idt,
            "EPSC": epsc,
        }
        for i in range(B)
    ]


def kernel(X_real, X_imag, norm_w, W, b):
    res = _run(_make_in_maps(X_real, X_imag, norm_w, W, b), trace=False)
    return np.stack([res.results[i]["out"] for i in range(B)]).astype(np.float32)


def kernel_profiled(X_real, X_imag, norm_w, W, b):
    res = _run(_make_in_maps(X_real, X_imag, norm_w, W, b), trace=True)
    out = np.stack([res.results[i]["out"] for i in range(B)]).astype(np.float32)
    return out, res


if __name__ == "__main__":
    print(f"NCHUNK={NCHUNK} NCOLB={NCOLB} NROWS={NROWS}")
    print(f"groups: {[(g[0], g[1], g[2], g[4]) for g in GROUPS]}")
    print(f"matmul segs per pass: {sum(len(s) for s in SEGS)}")
    per_part = (2 * NROWS * 4 + NCHUNK * TT * 4 + NCOLB * D * 4
                + 2 * NPLANE * F_BINS * 4 + 2 * 4 * D * 4 + 3 * 4 * D * 4) / 1024
    print(f"approx SBUF per partition: {per_part:.0f} KB")


# revision 30
# speedup vs baseline: 1.1981x; 1.0122x over previous
"""BandSplit Trainium2 kernel: 8-core data-parallel over batch.

out[b,t,n,d] = rsqrt(ms + eps) * (x_band @ (norm_w * W)) + bias
with ms = sum(x_band^2)/(4*fn),  x_band = contiguous freq slices of X.

Math restructure (exact):
  rsqrt(ms + eps) = sqrt(4fn) / sqrt(ssum),  ssum = sum(x^2) + 4*fn*eps
  out = (1/sqrt(ssum)) * (x @ W2 + sqrt(ssum) * b)     [bias-row trick]
  where W2[n,p,:] = sqrt(4fn) * norm_w[n,p] * W[n,p,:]

Per core (one batch element, T=512 tokens, 4 passes of 128):
  1. DMA X planes into natural layout (t part, f free).
  2. ssum per band via one fused multiply-reduce per band (eps as initial).
  3. Free-axis gather (on GpSimd) rearranges columns into the packed
     band-major row order; bands grouped by equal width give affine 3D
     APs, one copy per (plane, width-group). Bias slots get sqrt(ssum).
  4. PE-transpose each 128-column block -> packed row chunks (XtB, f32r).
  5. Per band: 1-2 float32r matmuls (K=4fn+1, M=128 tokens, N=384).
  6. Evict PSUM->SBUF scaled by 1/sqrt(ssum) per token; DMA out.
"""

import math
import numpy as np

# ---------------- problem geometry (hardcoded, matches reference) ----------
SR, N_FFT, D = 44100, 2048, 384
RANGES = [(1000, 2), (2000, 4), (4000, 12), (8000, 24), (16000, 48)]


def _compute_bands(sr=SR, n_fft=N_FFT):
    hz_per_bin = sr / n_fft
    max_bin = n_fft // 2 + 1
    boundaries = [0]
    for hi_hz, bins in RANGES:
        hi_bin = math.floor(hi_hz / hz_per_bin)
        while boundaries[-1] + bins <= hi_bin and boundaries[-1] + bins <= max_bin:
            boundaries.append(boundaries[-1] + bins)
    if boundaries[-1] < max_bin:
        remaining = max_bin - boundaries[-1]
        step = math.ceil(remaining / 6)
        b = boundaries[-1]
        while b + step < max_bin:
            b += step
            boundaries.append(b)
        boundaries.append(max_bin)
    return [(boundaries[i], boundaries[i + 1]) for i in range(len(boundaries) - 1)]


BANDS = _compute_bands()
N_BANDS = len(BANDS)
assert N_BANDS == 62
FN = [r - l for l, r in BANDS]
MAXF = max(FN)
F_BINS = N_FFT // 2 + 1  # 1025
EPS = 1e-8
B, C, T = 8, 2, 512
TT = 128  # tokens per pass (matmul M)
NP = T // TT  # 4 passes
NPLANE = 4  # (c, ri) combinations, g = 2*c + ri

# ---------------- regular row layout by equal-width band groups ------------
# logical row j of band n: j=0 -> bias; j=1+g*fn+k -> plane g, freq l+k.
# Bands with equal fn are consecutive; within a group each band's rows
# start at G_base + i*pad, giving affine gather patterns.


def _pad_for(rows):
    for p in (32, 64, 128, 256):
        if rows <= p:
            return p
    raise AssertionError


def _plan():
    groups = []  # (n0, k, fn, l0, pad, gbase)
    rowbase = 0
    n = 0
    while n < N_BANDS:
        fn = FN[n]
        k = 1
        while n + k < N_BANDS and FN[n + k] == fn:
            k += 1
        rows = 1 + 4 * fn
        pad = _pad_for(rows)
        gbase = rowbase
        rowbase += ((k * pad + 127) // 128) * 128
        groups.append((n, k, fn, BANDS[n][0], pad, gbase))
        n += k
    nrows = rowbase  # multiple of 128
    nchunk = nrows // 128

    band_base = {}
    for (n0, k, fn, l0, pad, gbase) in groups:
        for i in range(k):
            band_base[n0 + i] = gbase + i * pad

    # matmul segments per band: (chunk, row0, klen) covering 1+4fn rows
    segs0 = []
    for n in range(N_BANDS):
        rows = 1 + 4 * FN[n]
        bb = band_base[n]
        out = []
        while rows > 0:
            ch, r0 = bb // 128, bb % 128
            kl = min(rows, 128 - r0)
            out.append((ch, r0, kl))
            bb += kl
            rows -= kl
        segs0.append(out)

    # W2 column blocks: greedy interval packing of (row0, row0+klen),
    # largest-first so full-height segments claim blocks before slivers.
    allsegs = []
    for n in range(N_BANDS):
        for si, (ch, r0, kl) in enumerate(segs0[n]):
            allsegs.append((kl, n, si, ch, r0))
    allsegs.sort(key=lambda x: -x[0])
    colblocks = []
    cb_of = {}
    for (kl, n, si, ch, r0) in allsegs:
        for cbi in range(len(colblocks) + 1):
            if cbi == len(colblocks):
                colblocks.append([])
            ivs = colblocks[cbi]
            if all(e <= r0 or s >= r0 + kl for (s, e) in ivs):
                ivs.append((r0, r0 + kl))
                cb_of[(n, si)] = cbi
                break
    ncolb = len(colblocks)
    segs = []
    for n in range(N_BANDS):
        segs.append(
            [
                (ch, r0, kl, cb_of[(n, si)])
                for si, (ch, r0, kl) in enumerate(segs0[n])
            ]
        )

    # host W2 packing map: logical row j of band n -> (w2row, colblock)
    w2map = []
    for n in range(N_BANDS):
        rows = 1 + 4 * FN[n]
        m = []
        j = 0
        for (ch, r0, kl, cb) in segs[n]:
            for q in range(kl):
                m.append((r0 + q, cb))
            j += kl
        assert j == rows
        w2map.append(m)
    return groups, nchunk, ncolb, segs, band_base, w2map


GROUPS, NCHUNK, NCOLB, SEGS, BAND_BASE, W2MAP = _plan()
NROWS = NCHUNK * 128
CHUNK_GROUP = []
for ch in range(NCHUNK):
    gi = max(i for i, g in enumerate(GROUPS) if g[5] <= ch * 128)
    CHUNK_GROUP.append(gi)

# ---------------- host-side constant prep ---------------------------------


def _prep_consts(norm_w, W, b):
    import ml_dtypes
    w2sb = np.zeros((128, NCOLB * D), np.float32)
    for n in range(N_BANDS):
        fn = FN[n]
        s = math.sqrt(4.0 * fn)
        row, cb = W2MAP[n][0]
        w2sb[row, cb * D:(cb + 1) * D] = b[n]
        w2rows = (s * norm_w[n][:, None] * W[n]).astype(np.float32)  # (216, 384)
        for g in range(NPLANE):
            for k in range(fn):
                row, cb = W2MAP[n][1 + g * fn + k]
                w2sb[row, cb * D:(cb + 1) * D] = w2rows[g * MAXF + k]
    w2sb = w2sb.astype(ml_dtypes.bfloat16)
    idt = np.eye(128, dtype=np.float32)
    epsc = np.broadcast_to(
        (4.0 * np.asarray(FN, np.float64) * EPS).astype(np.float32)[None, :],
        (128, N_BANDS),
    ).copy()
    return w2sb, idt, epsc


# ---------------- bass kernel builder -------------------------------------

_BUILT = {}


def _build():
    if "nc" in _BUILT:
        return _BUILT["nc"]
    from contextlib import ExitStack
    import concourse.bacc as bacc
    import concourse.mybir as mybir
    from concourse import tile

    f32 = mybir.dt.float32
    f32r = mybir.dt.float32r
    mmdt = mybir.dt.bfloat16

    nc = bacc.Bacc(None, target_bir_lowering=False)
    x_re = nc.declare_dram_parameter("X_real", [C, T, F_BINS], f32, isOutput=False)
    x_im = nc.declare_dram_parameter("X_imag", [C, T, F_BINS], f32, isOutput=False)
    w2_e = nc.declare_dram_parameter("W2", [128, NCOLB * D], mmdt, isOutput=False)
    id_e = nc.declare_dram_parameter("IDT", [128, 128], f32, isOutput=False)
    eps_e = nc.declare_dram_parameter("EPSC", [128, N_BANDS], f32, isOutput=False)
    out_e = nc.declare_dram_parameter("out", [T, N_BANDS, D], f32, isOutput=True)

    GSZ = 8   # bands per output staging group
    WLD = 2   # W2 colblocks per staged load chunk

    with tile.TileContext(nc) as tc, ExitStack() as ctx:
        const = ctx.enter_context(tc.tile_pool(name="const", bufs=1))
        xtbp = ctx.enter_context(tc.tile_pool(name="xtbp", bufs=1))
        x4p = ctx.enter_context(tc.tile_pool(name="x4p", bufs=2))
        msp = ctx.enter_context(tc.tile_pool(name="msv", bufs=1))
        scr = ctx.enter_context(tc.tile_pool(name="scr", bufs=2))
        spool = ctx.enter_context(tc.tile_pool(name="stagep", bufs=3))
        trps = ctx.enter_context(tc.tile_pool(name="trp", bufs=3, space="PSUM"))
        mmps = ctx.enter_context(tc.tile_pool(name="mmp", bufs=5, space="PSUM"))

        idsb = const.tile([128, 128], f32)
        nc.sync.dma_start(out=idsb[:], in_=id_e[:])
        epsc = const.tile([128, N_BANDS], f32)
        nc.sync.dma_start(out=epsc[:], in_=eps_e[:])
        w2sb = const.tile([128, NCOLB * D], mmdt)

        # double-buffered per-group gather tiles
        xcat = [[], []]
        for bi in range(2):
            for gi, (n0, k, fn, l0, pad, gbase) in enumerate(GROUPS):
                gw = (GROUPS[gi + 1][5] - gbase) if gi + 1 < len(GROUPS) else (
                    NROWS - gbase
                )
                xg = const.tile(
                    [128, gw], f32, name=f"xcat{bi}_{gi}", tag=f"xcat{bi}_{gi}"
                )
                xcat[bi].append(xg)
                nc.gpsimd.memset(xg[:], 0.0)
        xtb = [
            xtbp.tile([128, TT], mmdt, name=f"xtb{m}", tag=f"xtb{m}")
            for m in range(NCHUNK)
        ]

        def phase_a(ps, gather_eng=None):
            """input DMA; half-freq band sums; per-group sqrt, gathers, bias."""
            t0 = ps * TT
            xb = xcat[ps % 2]
            geng = gather_eng or nc.vector
            x4 = [
                x4p.tile([128, F_BINS], f32, tag=f"x4_{g}", name=f"x4_{g}")
                for g in range(NPLANE)
            ]
            for g in range(NPLANE):
                xsrc = x_re if g % 2 == 0 else x_im
                nc.sync.dma_start(out=x4[g][:], in_=xsrc[g // 2, t0:t0 + TT, :])
            # squares accumulated in two frequency halves for finer deps
            FSPLIT = GROUPS[4][3]  # start freq of the fn=48 group
            halves = [(0, FSPLIT), (FSPLIT, F_BINS)]
            qh = [
                scr.tile([128, hi - lo], f32, tag=f"qh{hx}", name=f"qh{hx}")
                for hx, (lo, hi) in enumerate(halves)
            ]
            sbh = [
                scr.tile([128, hi - lo], f32, tag=f"sbh{hx}", name=f"sbh{hx}")
                for hx, (lo, hi) in enumerate(halves)
            ]
            for hx, (lo, hi) in enumerate(halves):
                for g in range(NPLANE):
                    dst = qh[hx] if g == 0 else sbh[hx]
                    nc.vector.tensor_tensor(
                        out=dst[:], in0=x4[g][:, lo:hi], in1=x4[g][:, lo:hi],
                        op=mybir.AluOpType.mult,
                    )
                    if g > 0:
                        nc.vector.tensor_tensor(
                            out=qh[hx][:], in0=qh[hx][:], in1=sbh[hx][:],
                            op=mybir.AluOpType.add,
                        )
            rs = msp.tile([128, N_BANDS], f32, tag=f"rs{ps}", name=f"rs{ps}")
            for gi, (n0, k, fn, l0, pad, gbase) in enumerate(GROUPS):
                kfn = k * fn
                hx = 0 if l0 < FSPLIT else 1
                qs = qh[hx][:, l0 - halves[hx][0]:l0 - halves[hx][0] + kfn]
                ssr = scr.tile([128, k], f32, tag=f"ssr{gi}", name=f"ssr{gi}")
                nc.vector.tensor_reduce(
                    out=ssr[:].rearrange("p (k o) -> p k o", o=1),
                    in_=qs.rearrange("p (k f) -> p k f", k=k),
                    op=mybir.AluOpType.add,
                    axis=mybir.AxisListType.X,
                )
                sqg = scr.tile([128, k], f32, tag=f"sqg{gi}", name=f"sqg{gi}")
                nc.vector.tensor_tensor(
                    out=ssr[:], in0=ssr[:], in1=epsc[:, n0:n0 + k],
                    op=mybir.AluOpType.add,
                )
                nc.scalar.activation(
                    out=sqg[:], in_=ssr[:],
                    func=mybir.ActivationFunctionType.Sqrt,
                )
                nc.vector.reciprocal(rs[:, n0:n0 + k], sqg[:])
                xg = xb[gi]
                for g in range(NPLANE):
                    src = x4[g][:, l0:l0 + kfn].rearrange("p (k f) -> p k f", k=k)
                    dst = xg[:, 0:k * pad].rearrange(
                        "p (k q) -> p k q", k=k
                    )[:, :, 1 + g * fn:1 + (g + 1) * fn]
                    geng.tensor_copy(dst, src)
                dstb = xg[:, 0:k * pad].rearrange(
                    "p (k q) -> p k q", k=k
                )[:, :, 0:1]
                geng.tensor_copy(
                    dstb, sqg[:].rearrange("p (k o) -> p k o", o=1)
                )
            return rs

        def phase_b(ps, rs):
            """transposes, per-band matmuls, scaled eviction, output DMA."""
            t0 = ps * TT
            xb = xcat[ps % 2]
            for ch in range(NCHUNK):
                gi = CHUNK_GROUP[ch]
                off = ch * 128 - GROUPS[gi][5]
                ptr = trps.tile([128, 128], f32, tag="trp")
                nc.tensor.transpose(ptr[:], xb[gi][:, off:off + 128], idsb[:])
                if ch % 2 == 0:
                    nc.vector.tensor_copy(xtb[ch][:], ptr[:])
                else:
                    nc.scalar.copy(xtb[ch][:], ptr[:])
            for n0 in range(0, N_BANDS, GSZ):
                gn = min(GSZ, N_BANDS - n0)
                stage = spool.tile([128, GSZ * D], f32, tag="stage")
                for n in range(n0, n0 + gn):
                    pmm = mmps.tile([128, D], f32, tag="mmp")
                    nseg = len(SEGS[n])
                    for si, (ch, row0, klen, cb) in enumerate(SEGS[n]):
                        nc.tensor.matmul(
                            pmm[:],
                            lhsT=xtb[ch][row0:row0 + klen, :],
                            rhs=w2sb[row0:row0 + klen, cb * D:(cb + 1) * D],
                            start=(si == 0),
                            stop=(si == nseg - 1),
                            tile_position=(row0, 0),
                        )
                    slot = stage[:, (n - n0) * D:(n - n0 + 1) * D]
                    if n % 2 == 0:
                        nc.vector.tensor_scalar_mul(slot, pmm[:], rs[:, n:n + 1])
                    else:
                        nc.scalar.mul(slot, pmm[:], rs[:, n:n + 1])
                nc.sync.dma_start(
                    out=out_e[t0:t0 + TT, n0:n0 + gn, :],
                    in_=stage[:, 0:gn * D].rearrange("p (n d) -> p n d", n=gn),
                )

        def load_w2():
            nc.sync.dma_start(out=w2sb[:], in_=w2_e[:])

        # software pipeline: A(0) w2 A(1) B(0) A(2) B(1) A(3) B(2) B(3)
        rs_of = {}
        rs_of[0] = phase_a(0, gather_eng=nc.vector)
        load_w2()
        rs_of[1] = phase_a(1)
        phase_b(0, rs_of[0])
        rs_of[2] = phase_a(2)
        phase_b(1, rs_of[1])
        rs_of[3] = phase_a(3)
        phase_b(2, rs_of[2])
        phase_b(3, rs_of[3])

    nc.finalize()
    _BUILT["nc"] = nc
    return nc


# ---------------- entry points --------------------------------------------


def _run(in_maps, trace=False):
    from concourse.bass_utils import run_bass_kernel_spmd

    nc = _build()
    return run_bass_kernel_spmd(nc, in_maps, core_ids=list(range(8)), trace=trace)


def _run_traced(in_maps, tmpdir=None):
    from concourse.bass_utils import run_bass_kernel_spmd

    nc = _build()
    return run_bass_kernel_spmd(
        nc, in_maps, core_ids=list(range(8)), trace=True, tmpdir=tmpdir
    )


def _make_in_maps(X_real, X_imag, norm_w, W, b):
    X_real = np.ascontiguousarray(np.asarray(X_real, np.float32))
    X_imag = np.ascontiguousarray(np.asarray(X_imag, np.float32))
    w2sb, idt, epsc = _prep_consts(
        np.asarray(norm_w, np.float32), np.asarray(W, np.float32),
        np.asarray(b, np.float32),
    )
    return [
        {
            "X_real": X_real[i],
            "X_imag": X_imag[i],
            "W2": w2sb,
            "IDT": idt,
            "EPSC": epsc,
        }
        for i in range(B)
    ]


def kernel(X_real, X_imag, norm_w, W, b):
    res = _run(_make_in_maps(X_real, X_imag, norm_w, W, b), trace=False)
    return np.stack([res.results[i]["out"] for i in range(B)]).astype(np.float32)


def kernel_profiled(X_real, X_imag, norm_w, W, b):
    res = _run(_make_in_maps(X_real, X_imag, norm_w, W, b), trace=True)
    out = np.stack([res.results[i]["out"] for i in range(B)]).astype(np.float32)
    return out, res


if __name__ == "__main__":
    print(f"NCHUNK={NCHUNK} NCOLB={NCOLB} NROWS={NROWS}")
    print(f"groups: {[(g[0], g[1], g[2], g[4]) for g in GROUPS]}")
    print(f"matmul segs per pass: {sum(len(s) for s in SEGS)}")
    per_part = (2 * NROWS * 4 + NCHUNK * TT * 4 + NCOLB * D * 4
                + 2 * NPLANE * F_BINS * 4 + 2 * 4 * D * 4 + 3 * 4 * D * 4) / 1024
    print(f"approx SBUF per partition: {per_part:.0f} KB")
